# revision 1
# baseline (speedup 1.0000x reference)
"""Trainium2 Bass kernel for nn_EnhancedAttentionLayer (B=4, S=2048, D=1024).

Single-head attention: Q/K/V projections -> scaled dot-product attention ->
output projection, fp32 in/out, computed with fp32r (TF32-like, 11-bit
mantissa) matmuls on the PE array (~227 ns sustained per 128x128x512 mm).

Sharding: 8 cores = (batch b in 0..3) x (query-half h in 0..1). Each core
computes Q for its 1024-query half, K/V for the full 2048-key batch element
(K/V projection duplicated across the pair - cross-core collectives work here
but hang under NTFF profiling, so they are not used), then scores/softmax/
context/out-proj for its queries.

All tensors are fed to the device PRE-TRANSPOSED by the host (numpy) so that
every matmul contraction dim lands on SBUF partitions with natural
(descriptor-friendly) DMA loads:
  xt  = x[b].T          [D, S]   (d on partitions; used for K and V)
  xq  = x[b].T half     [D, SQ]  (the core's query columns)
  w*t = W.T             [D, D]   ([in, out] layout)
Output is produced transposed (yt = y_half.T, [D_out, SQ]); the host
transposes back and reassembles.

Dataflow per core (all matmuls fp32r, moving dim 512):
  A1:  QT[e,q]   = wqt.T @ xq   (6-chain PSUM waves so the PE stays busy
                                 while the 8 MB wq+xq stream lands)
  A23: KT[e,k]   = wkt.T @ xt  and  V[s,e] = xt.T @ wvt -> DRAM scratch,
       one shared xt stream (SBUF can't hold KT+V+expT at once)
  B1:  ST[k,q]   = KT.T @ QT ; expT = exp(ST/32) (ACT, fused scale, f32r out)
       colsum    = ones128.T @ expT accumulated in PSUM - the ones MATRIX
       replicates the per-query sum across all 128 partitions, so the
       softmax denominator broadcast comes straight out of the matmul and
       reciprocal runs full-width (a [1,512] reciprocal crawls at 1/128 of
       DVE throughput and stalled the PE). Colsum for chunk k is emitted
       after chunk k+1's score chain so it never waits on ACT.
  B2:  ctxT[e,q] = V.T @ expT ; normalize by bcast (DVE) ;
       ytT[o,q]  = wot.T @ ctxT -> DRAM
Accumulation chains rotate through a 6-bank PSUM pool (consecutive chains
land in different banks, so drains overlap fills). vscr/yt stores issue
from the Scalar engine's HWDGE queue (idle in those phases) and v_ec
reloads from Sync - GpSimd SWDGE is left unused because its descriptor
generation contends with DVE's shared SBUF port during the PSUM-copy
traffic. The first xt quarter prefetches into kt_sb's tail columns
(written only by sh3's copies much later), so that load is ungated by
any pool-zone release and streams in during A1 compute.
softmax max-subtraction is skipped: scores ~ N(0,1), exp() is safe in fp32.
Biases are zeros by problem spec; bo is applied on host if nonzero.
"""
import sys

if '/opt/trn_rl_repo' not in sys.path:
    sys.path.insert(0, '/opt/trn_rl_repo')

from contextlib import ExitStack

import numpy as np

import concourse.bacc as bacc_mod
import concourse.mybir as mybir
import concourse.tile as tile
from concourse.bass_utils import run_bass_kernel_spmd

F32 = mybir.dt.float32
F32R = mybir.dt.float32r
EXP = mybir.ActivationFunctionType.Exp
MULT = mybir.AluOpType.mult

B, S, D = 4, 2048, 1024
SQ = 1024           # queries per core
P = 128
NDC = D // P        # 8 contraction chunks over d/e
NEC = D // P        # 8 output chunks over e/o
NKC = S // P        # 16 key chunks
NQH = SQ // 512     # 2 query column-halves (moving dim 512)
NSH = S // 512      # 4 key column-quarters

LAST_RESULT = [None]
_CACHE = {}


def build_nc():
    nc = bacc_mod.Bacc("TRN2", target_bir_lowering=False, debug=False)

    xt = nc.dram_tensor("xt", [D, S], F32R, kind="ExternalInput")
    xq = nc.dram_tensor("xq", [D, SQ], F32R, kind="ExternalInput")
    wqt = nc.dram_tensor("wqt", [D, D], F32R, kind="ExternalInput")
    wkt = nc.dram_tensor("wkt", [D, D], F32R, kind="ExternalInput")
    wvt = nc.dram_tensor("wvt", [D, D], F32R, kind="ExternalInput")
    wot = nc.dram_tensor("wot", [D, D], F32R, kind="ExternalInput")
    yt = nc.dram_tensor("yt", [D, SQ], F32, kind="ExternalOutput")
    vscr = nc.dram_tensor("vscr", [S, D], F32R)  # internal scratch

    def part3(ap):  # [R, C] dram -> [128, R/128, C] (rows on partitions)
        return ap.rearrange("(o i) c -> i o c", i=P)

    with tile.TileContext(nc) as tc, ExitStack() as ctx:
        pers = ctx.enter_context(tc.tile_pool(name="pers", bufs=1))
        ones_f = pers.tile([P, P], F32)
        nc.vector.memset(ones_f[:], 1.0)
        ones128 = pers.tile([P, P], F32R)
        nc.vector.tensor_copy(ones128[:], ones_f[:])
        bcast_sb = pers.tile([P, SQ], F32)

        # 6-bank PSUM rotation shared by every accumulation chain; +2 banks
        # for the two q-halves' colsum accumulators during B1
        mps = ctx.enter_context(tc.tile_pool(name="mps", bufs=6, space="PSUM"))

        with tc.tile_pool(name="qkt", bufs=1) as qkt:
            qt_sb = qkt.tile([P, NEC, SQ], F32R)   # 32 KB/part
            kt_sb = qkt.tile([P, NEC, S], F32R)    # 64 KB/part

            # wk pool coexists with a1 (zone stays clear of a1's release)
            with tc.tile_pool(name="wkp", bufs=1) as wkp:
                wk_sb = wkp.tile([P, NDC, D], F32R)

                # ---- A1: QT[e,q] = wqt.T @ xq ----
                with tc.tile_pool(name="a1", bufs=1) as a1:
                    wq_sb = a1.tile([P, NDC, D], F32R)
                    xq_sb = a1.tile([P, NDC, SQ], F32R)
                    for c in range(NDC):  # first chunks first: mm0 deps early
                        nc.sync.dma_start(wq_sb[:, c, :],
                                          wqt[c * P:(c + 1) * P, :])
                        nc.sync.dma_start(xq_sb[:, c, :],
                                          xq[c * P:(c + 1) * P, :])
                    # sh0's xt quarter lands in kt_sb's tail during A1
                    nc.sync.dma_start(kt_sb[:, :, 3 * 512:4 * 512],
                                      part3(xt[:, 0:512]))
                    for c in range(NDC):  # prefetch wk during A1
                        nc.sync.dma_start(wk_sb[:, c, :],
                                          wkt[c * P:(c + 1) * P, :])
                    # 16 chains (qh, ec) in waves of 6 so the PE has enough
                    # independent work to ride out the chunked-DMA arrival
                    chains = [(qh, ec) for qh in range(NQH)
                              for ec in range(NEC)]
                    for w0 in range(0, len(chains), 6):
                        wave = chains[w0:w0 + 6]
                        ps = [mps.tile([P, 512], F32, tag="ps",
                                       name=f"a1ps{w0}_{i}")
                              for i in range(len(wave))]
                        for dc in range(NDC):
                            for i, (qh, ec) in enumerate(wave):
                                nc.tensor.matmul(
                                    ps[i][:],
                                    wq_sb[:, dc, ec * P:(ec + 1) * P],
                                    xq_sb[:, dc, qh * 512:(qh + 1) * 512],
                                    start=(dc == 0), stop=(dc == NDC - 1))
                        for i, (qh, ec) in enumerate(wave):
                            nc.vector.tensor_copy(
                                qt_sb[:, ec, qh * 512:(qh + 1) * 512],
                                ps[i][:])

                # ---- A23: KT[e,k] and V[s,e] on one shared xt stream ----
                with tc.tile_pool(name="a23w", bufs=1) as a23w, \
                     tc.tile_pool(name="a23x", bufs=2) as a23x, \
                     tc.tile_pool(name="a23v", bufs=3) as a23v:
                    wv_sb = a23w.tile([P, NDC, D], F32R)
                    for sh in range(NSH):
                        if sh == 0:
                            xt_sh = kt_sb[:, :, 3 * 512:4 * 512]
                        else:
                            xt_sh = a23x.tile([P, NDC, 512], F32R, tag="xtsh")
                            nc.sync.dma_start(
                                xt_sh[:],
                                part3(xt[:, sh * 512:(sh + 1) * 512]))
                        if sh == 0:
                            # wv queued behind xt0 so the first KT wave
                            # isn't stuck behind a 4 MB weight load
                            for c in range(NDC):
                                nc.sync.dma_start(wv_sb[:, c, :],
                                                  wvt[c * P:(c + 1) * P, :])
                        # KT waves (ec quads)
                        for eg in range(2):
                            ps = [mps.tile([P, 512], F32, tag="ps",
                                           name=f"kps{sh}_{eg}_{i}")
                                  for i in range(4)]
                            for dc in range(NDC):
                                for e4 in range(4):
                                    ec = eg * 4 + e4
                                    nc.tensor.matmul(
                                        ps[e4][:],
                                        wk_sb[:, dc, ec * P:(ec + 1) * P],
                                        xt_sh[:, dc, :],
                                        start=(dc == 0), stop=(dc == NDC - 1))
                            for e4 in range(4):
                                ec = eg * 4 + e4
                                nc.vector.tensor_copy(
                                    kt_sb[:, ec, sh * 512:(sh + 1) * 512],
                                    ps[e4][:])
                        # V waves (sc pairs x 2 e-halves)
                        for scp in range(2):
                            ps = [mps.tile([P, 512], F32, tag="ps",
                                           name=f"vps{sh}_{scp}_{i}")
                                  for i in range(4)]
                            for dc in range(NDC):
                                for s2 in range(2):
                                    for eh in range(2):
                                        sc = scp * 2 + s2
                                        nc.tensor.matmul(
                                            ps[s2 * 2 + eh][:],
                                            xt_sh[:, dc, sc * P:(sc + 1) * P],
                                            wv_sb[:, dc,
                                                  eh * 512:(eh + 1) * 512],
                                            start=(dc == 0),
                                            stop=(dc == NDC - 1))
                            for s2 in range(2):
                                for eh in range(2):
                                    sc = scp * 2 + s2
                                    vst = a23v.tile([P, 512], F32R, tag="vst")
                                    nc.vector.tensor_copy(
                                        vst[:], ps[s2 * 2 + eh][:])
                                    r0 = (sh * 4 + sc) * P
                                    nc.scalar.dma_start(
                                        vscr[r0:r0 + P,
                                             eh * 512:(eh + 1) * 512],
                                        vst[:])

            # ---- B1: scoresT -> expT (+ colsums -> recip -> bcast) ----
            epool = ctx.enter_context(
                tc.tile_pool(name="expt", bufs=1, side="right"))
            expt_sb = epool.tile([P, NKC, SQ], F32R)  # 64 KB/part
            # v_ec prefetch pool opened early (right side) so B2's V loads
            # overlap B1 compute
            b2v = ctx.enter_context(
                tc.tile_pool(name="b2v", bufs=3, side="right"))
            with tc.tile_pool(name="sump", bufs=2, space="PSUM") as sump:
                for qh in range(NQH):
                    q0 = qh * 512
                    ps_sum = sump.tile([P, 512], F32, tag="pssum")
                    pending = None  # colsum for chunk k deferred one chain
                    for kc in range(NKC):
                        ps_s = mps.tile([P, 512], F32, tag="ps",
                                        name=f"pss{qh}_{kc}")
                        for ec in range(NEC):
                            nc.tensor.matmul(
                                ps_s[:], kt_sb[:, ec, kc * P:(kc + 1) * P],
                                qt_sb[:, ec, q0:q0 + 512],
                                start=(ec == 0), stop=(ec == NEC - 1))
                        nc.scalar.activation(
                            expt_sb[:, kc, q0:q0 + 512], ps_s[:], EXP,
                            scale=1.0 / 32.0)
                        if pending is not None:
                            nc.tensor.matmul(
                                ps_sum[:], ones128[:],
                                expt_sb[:, pending, q0:q0 + 512],
                                start=(pending == 0), stop=False)
                        pending = kc
                    nc.tensor.matmul(
                        ps_sum[:], ones128[:],
                        expt_sb[:, pending, q0:q0 + 512],
                        start=False, stop=True)
                    # sums replicated on every partition -> full-width recip
                    nc.vector.reciprocal(bcast_sb[:, q0:q0 + 512], ps_sum[:])

        # qkt pool closed; its space is reused by B2 pools (left side).
        # ---- B2: ctxT (normalized), then ytT = wot.T @ ctxT ----
        with tc.tile_pool(name="b2c", bufs=1) as b2c, \
             tc.tile_pool(name="b2w", bufs=3) as b2w, \
             tc.tile_pool(name="b2y", bufs=3) as b2y:
            ctx_sb = b2c.tile([P, NEC, SQ], F32R)
            for ec in range(NEC):
                v_ec = b2v.tile([P, NKC, P], F32R, tag="vec")
                nc.sync.dma_start(
                    v_ec[:],
                    vscr[:, ec * P:(ec + 1) * P].rearrange(
                        "(o i) e -> i o e", i=P))
                for qh in range(NQH):
                    q0 = qh * 512
                    ps_c = mps.tile([P, 512], F32, tag="ps",
                                    name=f"pc{ec}_{qh}")
                    for kc in range(NKC):
                        nc.tensor.matmul(
                            ps_c[:], v_ec[:, kc, :],
                            expt_sb[:, kc, q0:q0 + 512],
                            start=(kc == 0), stop=(kc == NKC - 1))
                    nc.vector.tensor_tensor(
                        ctx_sb[:, ec, q0:q0 + 512], ps_c[:],
                        bcast_sb[:, q0:q0 + 512], MULT)
            for oc in range(NEC):
                wo_oc = b2w.tile([P, NDC, P], F32R, tag="wo", name=f"wo{oc}")
                nc.sync.dma_start(
                    wo_oc[:],
                    part3(wot[:, oc * P:(oc + 1) * P]))
                for qh in range(NQH):
                    q0 = qh * 512
                    ps_o = mps.tile([P, 512], F32, tag="ps",
                                    name=f"po{oc}_{qh}")
                    for ec in range(NEC):
                        nc.tensor.matmul(
                            ps_o[:], wo_oc[:, ec, :],
                            ctx_sb[:, ec, q0:q0 + 512],
                            start=(ec == 0), stop=(ec == NEC - 1))
                    yst = b2y.tile([P, 512], F32, tag="yst")
                    nc.vector.tensor_copy(yst[:], ps_o[:])
                    nc.scalar.dma_start(
                        yt[oc * P:(oc + 1) * P, q0:q0 + 512], yst[:])

    nc.compile()
    return nc


def _get_nc():
    if "nc" not in _CACHE:
        _CACHE["nc"] = build_nc()
    return _CACHE["nc"]


def kernel(x, Wq, bq, Wk, bk, Wv, bv, Wo, bo, _trace=False):
    x = np.ascontiguousarray(np.asarray(x, dtype=np.float32))
    wqt = np.ascontiguousarray(np.asarray(Wq, dtype=np.float32).T)
    wkt = np.ascontiguousarray(np.asarray(Wk, dtype=np.float32).T)
    wvt = np.ascontiguousarray(np.asarray(Wv, dtype=np.float32).T)
    wot = np.ascontiguousarray(np.asarray(Wo, dtype=np.float32).T)

    in_maps = []
    xts = {}
    for c in range(8):
        b, h = c // 2, c % 2
        if b not in xts:
            xts[b] = np.ascontiguousarray(x[b].T)
        xt = xts[b]
        xq = np.ascontiguousarray(xt[:, h * SQ:(h + 1) * SQ])
        in_maps.append({"xt": xt, "xq": xq, "wqt": wqt, "wkt": wkt,
                        "wvt": wvt, "wot": wot})

    nc = _get_nc()
    kw = {}
    if _trace:
        kw = dict(trace=True, stitch_traces=False)
    res = run_bass_kernel_spmd(nc, in_maps, core_ids=list(range(8)), **kw)
    LAST_RESULT[0] = res

    y = np.empty((B, S, D), dtype=np.float32)
    for c in range(8):
        b, h = c // 2, c % 2
        y[b, h * SQ:(h + 1) * SQ, :] = res.results[c]["yt"].T

    bo = np.asarray(bo, dtype=np.float32)
    if bo.any():
        y = y + bo
    return y



# revision 4
# speedup vs baseline: 1.2730x; 1.2730x over previous
"""Trainium2 Bass kernel for nn_EnhancedAttentionLayer (B=4, S=2048, D=1024).

Single-head attention, fp32 in/out. Sharding: 8 cores = (batch b in 0..3) x
(query-half h in 0..1); each core produces the output rows for its 1024
queries. Two algebraic restructurings cut PE work ~20% vs the direct
Q/K/V/scores/ctx/out pipeline (1056 vs 1312 128x128x512-equivalent matmuls
per core), with no cross-core traffic:

  M-trick   scores = x^T (Wq^T Wk) x. Compute M = Wq^T Wk (128 MMs, weights
            only) and Zq = M^T xq (128), replacing Q-proj (128) + K-proj
            (256): K is never materialized.
  Late-V    ctx^T = Wv^T (xn^T expT) and y = Wo ctx, folded: precompute
            W2T = (Wo Wv)^T as Wv^T Wo^T (128 MMs, weights only), then
            G = xn^T @ expT (256) and y = W2T^T @ Gn (128), replacing
            V-proj (256) + ctx (256) + out-proj (128). The attention
            contraction (2048 keys) is applied to raw x BEFORE any weight
            projection, so projections only ever see 1024 columns.

Key-roll: each core's xt/xn have the key axis rotated so its own 1024 query
positions come first (attention is key-order invariant); xq is then always
xt cols 0:1024 -- no separate xq input, and the schedule is SPMD-identical
across cores.

Phase order M -> Zq -> W2T -> B1 -> B2 with a strictly-LIFO SBUF plan
(the tile allocator is a two-sided stack): left side carries
xta/zq + {wq,wk -> wv,wo} + M-scratch, right side carries xtb/expT/xn
opened late. x^T is split into xta (key cols 0:1024, streams behind wq/wk)
and xtb (cols 1024:2048, lands ~69us, needed ~100us). wv/wo reuse the
freed wq/wk bytes; their DMA is zone-gated only on M's last matmul (~30us)
and lands before W2T starts (~58us).

  A:  M = wq^T wk ; Zq = M^T @ xta ; W2T = wv^T wot
  B1: ST[k,q] = xt^T @ Zq ; expT = exp(ST/32) -> bf16 (ACT, fused scale)
      colsum via ones128 @ expT accumulated in PSUM (replicates the
      per-query sum across all partitions -> full-width reciprocal),
      emitted one chunk behind the score chains so it never waits on ACT
  B2: G[d,q] = xn^T @ expT (bf16); Gn = G * recip (DVE, PSUM drain)
      ytT[o,q] = W2T^T @ Gn -> DRAM
All matmuls 512-moving through a 6-bank PSUM rotation (+2 banks for B1
colsum accumulators). Numerics (vs fp32 reference, max-normalized): ~3.3e-3
simulated; bf16 is confined to weight products, expT and xn where
dot-product error stays ~0.4%; the scores/Zq/y chain stays fp32r.
Biases are zeros by problem spec; bo is applied on host if nonzero.
"""
import sys

if '/opt/trn_rl_repo' not in sys.path:
    sys.path.insert(0, '/opt/trn_rl_repo')

from contextlib import ExitStack

import numpy as np
import ml_dtypes

import concourse.bacc as bacc_mod
import concourse.mybir as mybir
import concourse.tile as tile
from concourse.bass_utils import run_bass_kernel_spmd

F32 = mybir.dt.float32
F32R = mybir.dt.float32r
BF16 = mybir.dt.bfloat16
EXP = mybir.ActivationFunctionType.Exp
MULT = mybir.AluOpType.mult

B, S, D = 4, 2048, 1024
SQ = 1024           # queries per core
P = 128
NDC = D // P        # 8 chunks of 128 over d / d' / e / o
NKC = S // P        # 16 key chunks
NQH = SQ // 512     # 2 query column-halves (moving dim 512)
NH2 = D // 512      # 2 column-halves of a [*, D] product

LAST_RESULT = [None]
_CACHE = {}


def build_nc():
    nc = bacc_mod.Bacc("TRN2", target_bir_lowering=False, debug=False)

    xt = nc.dram_tensor("xt", [D, S], F32R, kind="ExternalInput")
    xn = nc.dram_tensor("xn", [S, D], BF16, kind="ExternalInput")
    wq = nc.dram_tensor("wq", [D, D], BF16, kind="ExternalInput")
    wk = nc.dram_tensor("wk", [D, D], BF16, kind="ExternalInput")
    wv = nc.dram_tensor("wv", [D, D], BF16, kind="ExternalInput")
    wot = nc.dram_tensor("wot", [D, D], BF16, kind="ExternalInput")
    yt = nc.dram_tensor("yt", [D, SQ], F32, kind="ExternalOutput")

    def part3(ap):  # [R, C] dram -> [128, R/128, C] (rows on partitions)
        return ap.rearrange("(o i) c -> i o c", i=P)

    with tile.TileContext(nc) as tc, ExitStack() as ctx:
        pers = ctx.enter_context(tc.tile_pool(name="pers", bufs=1))
        ones_f = pers.tile([P, P], F32)
        nc.vector.memset(ones_f[:], 1.0)
        ones_bf = pers.tile([P, P], BF16)
        nc.vector.tensor_copy(ones_bf[:], ones_f[:])
        bcast_sb = pers.tile([P, SQ], F32)

        # 6-bank PSUM rotation shared by every accumulation chain; +2 banks
        # for the two q-halves' colsum accumulators during B1
        mps = ctx.enter_context(tc.tile_pool(name="mps", bufs=6, space="PSUM"))

        # live until B2: W2T = (Wo Wv)^T
        w2tp = ctx.enter_context(tc.tile_pool(name="w2t", bufs=1))
        w2t_sb = w2tp.tile([P, NDC, D], F32R)      # 32 KB/part

        def chain_waves(chains, lhs_of, rhs_of, out_of, nacc, tagbase):
            # waves of 6 chains; acc-step outer so 6 independent PSUM
            # accumulations ride out chunked-DMA arrival
            for w0 in range(0, len(chains), 6):
                wave = chains[w0:w0 + 6]
                ps = [mps.tile([P, 512], F32, tag="ps",
                               name=f"{tagbase}{w0}_{i}")
                      for i in range(len(wave))]
                for a in range(nacc):
                    for i, ch in enumerate(wave):
                        nc.tensor.matmul(ps[i][:], lhs_of(ch, a),
                                         rhs_of(ch, a),
                                         start=(a == 0), stop=(a == nacc - 1))
                for i, ch in enumerate(wave):
                    nc.vector.tensor_copy(out_of(ch), ps[i][:])

        chains = [(dc, h2) for dc in range(NDC) for h2 in range(NH2)]

        with tc.tile_pool(name="xta", bufs=1) as xtap, \
             tc.tile_pool(name="zqp", bufs=1) as zqp:
            xta_sb = xtap.tile([P, NDC, SQ], F32R)  # 32 KB/part, keys 0:1024
            zq_sb = zqp.tile([P, NDC, SQ], F32R)    # 32 KB/part

            # ---- A1: M = Wq^T Wk, then Zq = M^T @ xta ----
            with tc.tile_pool(name="wqk", bufs=1) as wqk:
                wq_sb = wqk.tile([P, NDC, D], BF16)
                wk_sb = wqk.tile([P, NDC, D], BF16)
                for c in range(NDC):  # first chunks first: mm0 deps early
                    nc.sync.dma_start(wq_sb[:, c, :],
                                      wq[c * P:(c + 1) * P, :])
                    nc.sync.dma_start(wk_sb[:, c, :],
                                      wk[c * P:(c + 1) * P, :])
                # query-half of x^T streams behind wq/wk (Zq needs it ~30us)
                for sh in range(2):
                    nc.sync.dma_start(
                        xta_sb[:, :, sh * 512:(sh + 1) * 512],
                        part3(xt[:, sh * 512:(sh + 1) * 512]))

                with tc.tile_pool(name="mp", bufs=1) as mp:
                    m_sb = mp.tile([P, NDC, D], F32R)   # 32 KB/part
                    # M[d, d'] = sum_e1 Wq[e1, d] Wk[e1, d']
                    chain_waves(
                        chains,
                        lambda ch, a: wq_sb[:, a, ch[0] * P:(ch[0] + 1) * P],
                        lambda ch, a: wk_sb[:, a,
                                            ch[1] * 512:(ch[1] + 1) * 512],
                        lambda ch: m_sb[:, ch[0],
                                        ch[1] * 512:(ch[1] + 1) * 512],
                        NDC, "mm")
                    # Zq[d', q] = sum_d M[d, d'] xq[d, q]; xq = xta
                    chain_waves(
                        chains,
                        lambda ch, a: m_sb[:, a, ch[0] * P:(ch[0] + 1) * P],
                        lambda ch, a: xta_sb[:, a,
                                             ch[1] * 512:(ch[1] + 1) * 512],
                        lambda ch: zq_sb[:, ch[0],
                                         ch[1] * 512:(ch[1] + 1) * 512],
                        NDC, "zq")

            # ---- A2: W2T = Wv^T Wo^T (wv/wo reuse wq/wk's bytes; DMA is
            # zone-gated on M's last matmul and lands before W2T starts) ----
            with tc.tile_pool(name="wvo", bufs=1) as wvo:
                wv_sb = wvo.tile([P, NDC, D], BF16)
                wo_sb = wvo.tile([P, NDC, D], BF16)
                for c in range(NDC):
                    nc.sync.dma_start(wv_sb[:, c, :],
                                      wv[c * P:(c + 1) * P, :])
                    nc.sync.dma_start(wo_sb[:, c, :],
                                      wot[c * P:(c + 1) * P, :])
                # key cols 1024:2048 of x^T -> right side, lands ~69us,
                # first needed by B1's kc=8 chain ~100us
                xtbp = ctx.enter_context(
                    tc.tile_pool(name="xtb", bufs=1, side="right"))
                xtb_sb = xtbp.tile([P, NDC, SQ], F32R)  # 32 KB/part
                for sh in range(2):
                    nc.sync.dma_start(
                        xtb_sb[:, :, sh * 512:(sh + 1) * 512],
                        part3(xt[:, SQ + sh * 512:SQ + (sh + 1) * 512]))
                # W2T[d, o] = sum_e Wv[e, d] Wo[o, e]
                chain_waves(
                    chains,
                    lambda ch, a: wv_sb[:, a, ch[0] * P:(ch[0] + 1) * P],
                    lambda ch, a: wo_sb[:, a,
                                        ch[1] * 512:(ch[1] + 1) * 512],
                    lambda ch: w2t_sb[:, ch[0],
                                      ch[1] * 512:(ch[1] + 1) * 512],
                    NDC, "w2")

            # ---- B1: scoresT -> expT (+ colsums -> recip -> bcast) ----
            epool = ctx.enter_context(
                tc.tile_pool(name="expt", bufs=1, side="right"))
            expt_sb = epool.tile([P, NKC, SQ], BF16)   # 32 KB/part
            xnp = ctx.enter_context(
                tc.tile_pool(name="xnp", bufs=1, side="right"))
            xn_sb = xnp.tile([P, NKC, D], BF16)        # 32 KB/part
            nc.sync.dma_start(xn_sb[:], part3(xn))     # lands during B1
            with tc.tile_pool(name="sump", bufs=2, space="PSUM") as sump:
                for qh in range(NQH):
                    q0 = qh * 512
                    ps_sum = sump.tile([P, 512], F32, tag="pssum")
                    pending = None  # colsum for chunk k deferred one chain
                    for kc in range(NKC):
                        xk = (xta_sb[:, :, kc * P:(kc + 1) * P]
                              if kc < NDC else
                              xtb_sb[:, :, (kc - NDC) * P:(kc - NDC + 1) * P])
                        ps_s = mps.tile([P, 512], F32, tag="ps",
                                        name=f"pss{qh}_{kc}")
                        for dc in range(NDC):
                            nc.tensor.matmul(
                                ps_s[:], xk[:, dc, :],
                                zq_sb[:, dc, q0:q0 + 512],
                                start=(dc == 0), stop=(dc == NDC - 1))
                        nc.scalar.activation(
                            expt_sb[:, kc, q0:q0 + 512], ps_s[:], EXP,
                            scale=1.0 / 32.0)
                        if pending is not None:
                            nc.tensor.matmul(
                                ps_sum[:], ones_bf[:],
                                expt_sb[:, pending, q0:q0 + 512],
                                start=(pending == 0), stop=False)
                        pending = kc
                    nc.tensor.matmul(
                        ps_sum[:], ones_bf[:],
                        expt_sb[:, pending, q0:q0 + 512],
                        start=False, stop=True)
                    # sums replicated on every partition -> full-width recip
                    nc.vector.reciprocal(bcast_sb[:, q0:q0 + 512], ps_sum[:])

        # xta/zq closed; B2 reuses their space (left side)
        # ---- B2: G = xn^T @ expT, normalized; ytT = W2T^T @ Gn ----
        with tc.tile_pool(name="gp", bufs=1) as gp, \
             tc.tile_pool(name="yp", bufs=3) as yp:
            g_sb = gp.tile([P, NDC, SQ], F32R)
            for qh in range(NQH):
                q0 = qh * 512
                for dc in range(NDC):
                    ps_g = mps.tile([P, 512], F32, tag="ps",
                                    name=f"pg{qh}_{dc}")
                    for kc in range(NKC):
                        nc.tensor.matmul(
                            ps_g[:], xn_sb[:, kc, dc * P:(dc + 1) * P],
                            expt_sb[:, kc, q0:q0 + 512],
                            start=(kc == 0), stop=(kc == NKC - 1))
                    nc.vector.tensor_tensor(
                        g_sb[:, dc, q0:q0 + 512], ps_g[:],
                        bcast_sb[:, q0:q0 + 512], MULT)
            for qh in range(NQH):
                q0 = qh * 512
                for oc in range(NDC):
                    ps_y = mps.tile([P, 512], F32, tag="ps",
                                    name=f"py{qh}_{oc}")
                    for dc in range(NDC):
                        nc.tensor.matmul(
                            ps_y[:], w2t_sb[:, dc, oc * P:(oc + 1) * P],
                            g_sb[:, dc, q0:q0 + 512],
                            start=(dc == 0), stop=(dc == NDC - 1))
                    yst = yp.tile([P, 512], F32, tag="yst")
                    nc.vector.tensor_copy(yst[:], ps_y[:])
                    nc.scalar.dma_start(
                        yt[oc * P:(oc + 1) * P, q0:q0 + 512], yst[:])

    nc.compile()
    return nc


def _get_nc():
    if "nc" not in _CACHE:
        _CACHE["nc"] = build_nc()
    return _CACHE["nc"]


def kernel(x, Wq, bq, Wk, bk, Wv, bv, Wo, bo, _trace=False):
    x = np.asarray(x, dtype=np.float32)
    bf = ml_dtypes.bfloat16
    wq_b = np.ascontiguousarray(np.asarray(Wq, dtype=np.float32)).astype(bf)
    wk_b = np.ascontiguousarray(np.asarray(Wk, dtype=np.float32)).astype(bf)
    wv_b = np.ascontiguousarray(np.asarray(Wv, dtype=np.float32)).astype(bf)
    wot_b = np.ascontiguousarray(
        np.asarray(Wo, dtype=np.float32).T).astype(bf)

    in_maps = []
    for c in range(8):
        b, h = c // 2, c % 2
        xb = x[b]
        if h == 0:
            xroll = xb
        else:
            xroll = np.concatenate([xb[SQ:], xb[:SQ]], axis=0)
        xt = np.ascontiguousarray(xroll.T)
        xnb = np.ascontiguousarray(xroll).astype(bf)
        in_maps.append({"xt": xt, "xn": xnb, "wq": wq_b, "wk": wk_b,
                        "wv": wv_b, "wot": wot_b})

    nc = _get_nc()
    kw = {}
    if _trace:
        kw = dict(trace=True, stitch_traces=False)
    res = run_bass_kernel_spmd(nc, in_maps, core_ids=list(range(8)), **kw)
    LAST_RESULT[0] = res

    y = np.empty((B, S, D), dtype=np.float32)
    for c in range(8):
        b, h = c // 2, c % 2
        y[b, h * SQ:(h + 1) * SQ, :] = res.results[c]["yt"].T

    bo = np.asarray(bo, dtype=np.float32)
    if bo.any():
        y = y + bo
    return y


# revision 7
# speedup vs baseline: 1.2808x; 1.0061x over previous
"""Trainium2 Bass kernel for nn_EnhancedAttentionLayer (B=4, S=2048, D=1024).

Single-head attention, fp32 in/out. Sharding: 8 cores = (batch b in 0..3) x
(query-half h in 0..1); each core produces the output rows for its 1024
queries. Two algebraic restructurings cut PE work ~20% vs the direct
Q/K/V/scores/ctx/out pipeline (1056 vs 1312 128x128x512-equivalent matmuls
per core), with no cross-core traffic:

  M-trick   scores = x^T (Wq^T Wk) x. Compute M = Wq^T Wk (128 MMs, weights
            only) and Zq = M^T xq (128), replacing Q-proj (128) + K-proj
            (256): K is never materialized.
  Late-V    ctx^T = Wv^T (xn^T expT) and y = Wo ctx, folded: precompute
            W2T = (Wo Wv)^T as Wv^T Wo^T (128 MMs, weights only), then
            G = xn^T @ expT (256) and y = W2T^T @ Gn (128), replacing
            V-proj (256) + ctx (256) + out-proj (128). The attention
            contraction (2048 keys) is applied to raw x BEFORE any weight
            projection, so projections only ever see 1024 columns.

Key-roll: each core's xt/xn have the key axis rotated so its own 1024 query
positions come first (attention is key-order invariant); xq is then always
xt cols 0:1024 -- no separate xq input, and the schedule is SPMD-identical
across cores.

Phase order M -> Zq -> W2T -> B1 -> B2 with a strictly-LIFO SBUF plan
(the tile allocator is a two-sided stack): left side carries
xta/zq + {wq,wk -> wv,wo} + M-scratch, right side carries xtb/expT/xn
opened late. x^T is split into xta (key cols 0:1024, streams behind wq/wk)
and xtb (cols 1024:2048, lands ~69us, needed ~100us). wv/wo reuse the
freed wq/wk bytes; their DMA is zone-gated only on M's last matmul (~30us)
and lands before W2T starts (~58us).

  A:  M = wq^T wk ; Zq = M^T @ xta ; W2T = wv^T wot
  B1: ST[k,q] = xt^T @ Zq ; expT = exp(ST/32) -> bf16 (ACT, fused scale)
      colsum via ones128 @ expT accumulated in PSUM (replicates the
      per-query sum across all partitions -> full-width reciprocal),
      emitted one chunk behind the score chains so it never waits on ACT
  B2: G[d,q] = xn^T @ expT (bf16); Gn = G * recip (DVE, PSUM drain)
      ytT[o,q] = W2T^T @ Gn -> DRAM
All matmuls 512-moving through a 6-bank PSUM rotation (+2 banks for B1
colsum accumulators). Numerics (vs fp32 reference, max-normalized): ~3.3e-3
simulated; bf16 is confined to weight products, expT and xn where
dot-product error stays ~0.4%; the scores/Zq/y chain stays fp32r.
Biases are zeros by problem spec; bo is applied on host if nonzero.
"""
import sys

if '/opt/trn_rl_repo' not in sys.path:
    sys.path.insert(0, '/opt/trn_rl_repo')

from contextlib import ExitStack

import numpy as np
import ml_dtypes

import concourse.bacc as bacc_mod
import concourse.mybir as mybir
import concourse.tile as tile
from concourse.bass_utils import run_bass_kernel_spmd

F32 = mybir.dt.float32
F32R = mybir.dt.float32r
BF16 = mybir.dt.bfloat16
EXP = mybir.ActivationFunctionType.Exp
MULT = mybir.AluOpType.mult

B, S, D = 4, 2048, 1024
SQ = 1024           # queries per core
P = 128
NDC = D // P        # 8 chunks of 128 over d / d' / e / o
NKC = S // P        # 16 key chunks
NQH = SQ // 512     # 2 query column-halves (moving dim 512)
NH2 = D // 512      # 2 column-halves of a [*, D] product

LAST_RESULT = [None]
_CACHE = {}


def build_nc():
    nc = bacc_mod.Bacc("TRN2", target_bir_lowering=False, debug=False)

    xt = nc.dram_tensor("xt", [D, S], F32R, kind="ExternalInput")
    xn = nc.dram_tensor("xn", [S, D], BF16, kind="ExternalInput")
    wq = nc.dram_tensor("wq", [D, D], BF16, kind="ExternalInput")
    wk = nc.dram_tensor("wk", [D, D], BF16, kind="ExternalInput")
    wv = nc.dram_tensor("wv", [D, D], BF16, kind="ExternalInput")
    wot = nc.dram_tensor("wot", [D, D], BF16, kind="ExternalInput")
    yt = nc.dram_tensor("yt", [D, SQ], F32, kind="ExternalOutput")

    def part3(ap):  # [R, C] dram -> [128, R/128, C] (rows on partitions)
        return ap.rearrange("(o i) c -> i o c", i=P)

    with tile.TileContext(nc) as tc, ExitStack() as ctx:
        pers = ctx.enter_context(tc.tile_pool(name="pers", bufs=1))
        ones_f = pers.tile([P, P], F32)
        nc.vector.memset(ones_f[:], 1.0)
        ones_bf = pers.tile([P, P], BF16)
        nc.vector.tensor_copy(ones_bf[:], ones_f[:])
        bcast_sb = pers.tile([P, SQ], F32)
        warm_sb = pers.tile([P, P], F32)

        # 6-bank PSUM rotation shared by every accumulation chain; +2 banks
        # for the two q-halves' colsum accumulators during B1
        mps = ctx.enter_context(tc.tile_pool(name="mps", bufs=6, space="PSUM"))

        # ~4.3us of dummy matmuls on ones_bf while the first weight chunks
        # stream in: keeps the PE busy from ~8us so the HAM throttle reaches
        # K=8/8 before the first real matmul instead of ~5us into M
        ps_w = mps.tile([P, P], F32, tag="ps", name="warm")
        for i in range(40):
            nc.tensor.matmul(ps_w[:], ones_bf[:], ones_bf[:],
                             start=(i == 0), stop=(i == 39))
        nc.vector.tensor_copy(warm_sb[:], ps_w[:])

        # live until B2: W2T = (Wo Wv)^T
        w2tp = ctx.enter_context(tc.tile_pool(name="w2t", bufs=1))
        w2t_sb = w2tp.tile([P, NDC, D], F32R)      # 32 KB/part

        def chain_waves(chains, lhs_of, rhs_of, out_of, nacc, tagbase):
            # waves of 6 chains; acc-step outer so 6 independent PSUM
            # accumulations ride out chunked-DMA arrival
            for w0 in range(0, len(chains), 6):
                wave = chains[w0:w0 + 6]
                ps = [mps.tile([P, 512], F32, tag="ps",
                               name=f"{tagbase}{w0}_{i}")
                      for i in range(len(wave))]
                for a in range(nacc):
                    for i, ch in enumerate(wave):
                        nc.tensor.matmul(ps[i][:], lhs_of(ch, a),
                                         rhs_of(ch, a),
                                         start=(a == 0), stop=(a == nacc - 1))
                for i, ch in enumerate(wave):
                    nc.vector.tensor_copy(out_of(ch), ps[i][:])

        chains = [(dc, h2) for dc in range(NDC) for h2 in range(NH2)]

        with tc.tile_pool(name="xta", bufs=1) as xtap, \
             tc.tile_pool(name="zqp", bufs=1) as zqp:
            xta_sb = xtap.tile([P, NDC, SQ], F32R)  # 32 KB/part, keys 0:1024
            zq_sb = zqp.tile([P, NDC, SQ], F32R)    # 32 KB/part

            # ---- A1: M = Wq^T Wk, then Zq = M^T @ xta ----
            with tc.tile_pool(name="wqk", bufs=1) as wqk:
                wq_sb = wqk.tile([P, NDC, D], BF16)
                wk_sb = wqk.tile([P, NDC, D], BF16)
                for c in range(NDC):  # wq on sync, wk on scalar: chunk
                    nc.sync.dma_start(wq_sb[:, c, :],  # pairs land 2x faster
                                      wq[c * P:(c + 1) * P, :])
                    nc.scalar.dma_start(wk_sb[:, c, :],
                                        wk[c * P:(c + 1) * P, :])
                # query-half of x^T streams behind wq/wk (Zq needs it ~40us)
                nc.sync.dma_start(xta_sb[:, :, 0:512], part3(xt[:, 0:512]))
                nc.scalar.dma_start(xta_sb[:, :, 512:1024],
                                    part3(xt[:, 512:1024]))

                with tc.tile_pool(name="mp", bufs=1) as mp:
                    m_sb = mp.tile([P, NDC, D], F32R)   # 32 KB/part
                    # M[d, d'] = sum_e1 Wq[e1, d] Wk[e1, d']
                    chain_waves(
                        chains,
                        lambda ch, a: wq_sb[:, a, ch[0] * P:(ch[0] + 1) * P],
                        lambda ch, a: wk_sb[:, a,
                                            ch[1] * 512:(ch[1] + 1) * 512],
                        lambda ch: m_sb[:, ch[0],
                                        ch[1] * 512:(ch[1] + 1) * 512],
                        NDC, "mm")
                    # Zq[d', q] = sum_d M[d, d'] xq[d, q]; xq = xta
                    chain_waves(
                        chains,
                        lambda ch, a: m_sb[:, a, ch[0] * P:(ch[0] + 1) * P],
                        lambda ch, a: xta_sb[:, a,
                                             ch[1] * 512:(ch[1] + 1) * 512],
                        lambda ch: zq_sb[:, ch[0],
                                         ch[1] * 512:(ch[1] + 1) * 512],
                        NDC, "zq")

            # ---- A2: W2T = Wv^T Wo^T (wv/wo reuse wq/wk's bytes; DMA is
            # zone-gated on M's last matmul and lands before W2T starts) ----
            with tc.tile_pool(name="wvo", bufs=1) as wvo:
                wv_sb = wvo.tile([P, NDC, D], BF16)
                wo_sb = wvo.tile([P, NDC, D], BF16)
                for c in range(NDC):
                    nc.sync.dma_start(wv_sb[:, c, :],
                                      wv[c * P:(c + 1) * P, :])
                    nc.scalar.dma_start(wo_sb[:, c, :],
                                        wot[c * P:(c + 1) * P, :])
                # key cols 1024:2048 of x^T -> right side, lands ~69us,
                # first needed by B1's kc=8 chain ~100us
                xtbp = ctx.enter_context(
                    tc.tile_pool(name="xtb", bufs=1, side="right"))
                xtb_sb = xtbp.tile([P, NDC, SQ], F32R)  # 32 KB/part
                for sh in range(2):
                    nc.sync.dma_start(
                        xtb_sb[:, :, sh * 512:(sh + 1) * 512],
                        part3(xt[:, SQ + sh * 512:SQ + (sh + 1) * 512]))
                # W2T[d, o] = sum_e Wv[e, d] Wo[o, e]
                chain_waves(
                    chains,
                    lambda ch, a: wv_sb[:, a, ch[0] * P:(ch[0] + 1) * P],
                    lambda ch, a: wo_sb[:, a,
                                        ch[1] * 512:(ch[1] + 1) * 512],
                    lambda ch: w2t_sb[:, ch[0],
                                      ch[1] * 512:(ch[1] + 1) * 512],
                    NDC, "w2")

            # ---- B1: scoresT -> expT (+ colsums -> recip -> bcast) ----
            epool = ctx.enter_context(
                tc.tile_pool(name="expt", bufs=1, side="right"))
            expt_sb = epool.tile([P, NKC, SQ], BF16)   # 32 KB/part
            xnp = ctx.enter_context(
                tc.tile_pool(name="xnp", bufs=1, side="right"))
            xn_sb = xnp.tile([P, NKC, D], BF16)        # 32 KB/part
            nc.sync.dma_start(xn_sb[:], part3(xn))     # lands during B1
            with tc.tile_pool(name="sump", bufs=2, space="PSUM") as sump:
                for qh in range(NQH):
                    q0 = qh * 512
                    ps_sum = sump.tile([P, 512], F32, tag="pssum")
                    pending = None  # colsum for chunk k deferred one chain
                    for kc in range(NKC):
                        xk = (xta_sb[:, :, kc * P:(kc + 1) * P]
                              if kc < NDC else
                              xtb_sb[:, :, (kc - NDC) * P:(kc - NDC + 1) * P])
                        ps_s = mps.tile([P, 512], F32, tag="ps",
                                        name=f"pss{qh}_{kc}")
                        for dc in range(NDC):
                            nc.tensor.matmul(
                                ps_s[:], xk[:, dc, :],
                                zq_sb[:, dc, q0:q0 + 512],
                                start=(dc == 0), stop=(dc == NDC - 1))
                        nc.scalar.activation(
                            expt_sb[:, kc, q0:q0 + 512], ps_s[:], EXP,
                            scale=1.0 / 32.0)
                        if pending is not None:
                            nc.tensor.matmul(
                                ps_sum[:], ones_bf[:],
                                expt_sb[:, pending, q0:q0 + 512],
                                start=(pending == 0), stop=False)
                        pending = kc
                    nc.tensor.matmul(
                        ps_sum[:], ones_bf[:],
                        expt_sb[:, pending, q0:q0 + 512],
                        start=False, stop=True)
                    # sums replicated on every partition -> full-width recip
                    nc.vector.reciprocal(bcast_sb[:, q0:q0 + 512], ps_sum[:])

        # xta/zq closed; B2 reuses their space (left side)
        # ---- B2: G = xn^T @ expT, normalized; ytT = W2T^T @ Gn ----
        with tc.tile_pool(name="gp", bufs=1) as gp, \
             tc.tile_pool(name="yp", bufs=3) as yp:
            g_sb = gp.tile([P, NDC, SQ], F32R)
            for qh in range(NQH):
                q0 = qh * 512
                for dc in range(NDC):
                    ps_g = mps.tile([P, 512], F32, tag="ps",
                                    name=f"pg{qh}_{dc}")
                    for kc in range(NKC):
                        nc.tensor.matmul(
                            ps_g[:], xn_sb[:, kc, dc * P:(dc + 1) * P],
                            expt_sb[:, kc, q0:q0 + 512],
                            start=(kc == 0), stop=(kc == NKC - 1))
                    nc.vector.tensor_tensor(
                        g_sb[:, dc, q0:q0 + 512], ps_g[:],
                        bcast_sb[:, q0:q0 + 512], MULT)
            for qh in range(NQH):
                q0 = qh * 512
                for oc in range(NDC):
                    ps_y = mps.tile([P, 512], F32, tag="ps",
                                    name=f"py{qh}_{oc}")
                    for dc in range(NDC):
                        nc.tensor.matmul(
                            ps_y[:], w2t_sb[:, dc, oc * P:(oc + 1) * P],
                            g_sb[:, dc, q0:q0 + 512],
                            start=(dc == 0), stop=(dc == NDC - 1))
                    yst = yp.tile([P, 512], F32, tag="yst")
                    nc.vector.tensor_copy(yst[:], ps_y[:])
                    nc.scalar.dma_start(
                        yt[oc * P:(oc + 1) * P, q0:q0 + 512], yst[:])

    nc.compile()
    return nc


def _get_nc():
    if "nc" not in _CACHE:
        _CACHE["nc"] = build_nc()
    return _CACHE["nc"]


def kernel(x, Wq, bq, Wk, bk, Wv, bv, Wo, bo, _trace=False):
    x = np.asarray(x, dtype=np.float32)
    bf = ml_dtypes.bfloat16
    wq_b = np.ascontiguousarray(np.asarray(Wq, dtype=np.float32)).astype(bf)
    wk_b = np.ascontiguousarray(np.asarray(Wk, dtype=np.float32)).astype(bf)
    wv_b = np.ascontiguousarray(np.asarray(Wv, dtype=np.float32)).astype(bf)
    wot_b = np.ascontiguousarray(
        np.asarray(Wo, dtype=np.float32).T).astype(bf)

    in_maps = []
    for c in range(8):
        b, h = c // 2, c % 2
        xb = x[b]
        if h == 0:
            xroll = xb
        else:
            xroll = np.concatenate([xb[SQ:], xb[:SQ]], axis=0)
        xt = np.ascontiguousarray(xroll.T)
        xnb = np.ascontiguousarray(xroll).astype(bf)
        in_maps.append({"xt": xt, "xn": xnb, "wq": wq_b, "wk": wk_b,
                        "wv": wv_b, "wot": wot_b})

    nc = _get_nc()
    kw = {}
    if _trace:
        kw = dict(trace=True, stitch_traces=False)
    res = run_bass_kernel_spmd(nc, in_maps, core_ids=list(range(8)), **kw)
    LAST_RESULT[0] = res

    y = np.empty((B, S, D), dtype=np.float32)
    for c in range(8):
        b, h = c // 2, c % 2
        y[b, h * SQ:(h + 1) * SQ, :] = res.results[c]["yt"].T

    bo = np.asarray(bo, dtype=np.float32)
    if bo.any():
        y = y + bo
    return y


# revision 8
# speedup vs baseline: 1.3259x; 1.0352x over previous
"""Trainium2 Bass kernel for nn_EnhancedAttentionLayer (B=4, S=2048, D=1024).

Single-head attention, fp32 in/out. Sharding: 8 cores = (batch b in 0..3) x
(query-half h in 0..1); each core produces the output rows for its 1024
queries. Two algebraic restructurings cut PE work ~20% vs the direct
Q/K/V/scores/ctx/out pipeline (1056 vs 1312 128x128x512-equivalent matmuls
per core), with no cross-core traffic:

  M-trick   scores = x^T (Wq^T Wk) x. Compute M = Wq^T Wk (128 MMs, weights
            only) and Zq = M^T xq (128), replacing Q-proj (128) + K-proj
            (256): K is never materialized.
  Late-V    ctx^T = Wv^T (xn^T expT) and y = Wo ctx, folded: precompute
            W2T = (Wo Wv)^T as Wv^T Wo^T (128 MMs, weights only), then
            G = xn^T @ expT (256) and y = W2T^T @ Gn (128), replacing
            V-proj (256) + ctx (256) + out-proj (128). The attention
            contraction (2048 keys) is applied to raw x BEFORE any weight
            projection, so projections only ever see 1024 columns.

Key-roll: each core's xt/xn have the key axis rotated so its own 1024 query
positions come first (attention is key-order invariant); xq is then always
xt cols 0:1024 -- no separate xq input, and the schedule is SPMD-identical
across cores.

All matmul operands are bf16 (PSUM accumulates fp32): same PE rate as
fp32r at 512-moving, but LDWEIGHTS runs FWL (~107ns vs 187ns fp32-HIGH),
which is what held fp32r chains at ~227ns/MM instead of 216. Intermediates
(M, Zq, W2T, expT, G) are drained from PSUM straight to bf16. Measured
error vs the fp32 reference is ~8e-3 max-normalized (gate 2e-2); the
fp32r variant of this same kernel measured 3.45e-3 at +7us.

Phases: warmup (40 dummy matmuls on ones while the first weight chunks
stream in, so the HAM throttle hits K=8/8 before the first real MM),
  A:  M = wq^T wk ; Zq = M^T @ xq ; W2T = wv^T wot
  B1: ST[k,q] = xt^T @ Zq ; expT = exp(ST/32) -> bf16 (ACT, fused scale)
      colsum via ones128 @ expT accumulated in PSUM (replicates the
      per-query sum across all partitions -> full-width reciprocal),
      emitted one chunk behind the score chains so it never waits on ACT
  B2: G[d,q] = xn^T @ expT ; Gn = G * recip (DVE, PSUM drain)
      ytT[o,q] = W2T^T @ Gn -> DRAM
512-moving matmuls through a 6-bank PSUM rotation (+2 banks for B1 colsum
accumulators). Weight loads are split across the Sync and Scalar HWDGE
queues so chunk pairs land 2x faster; wv/wo reuse wq/wk's SBUF bytes
(zone-gated on M's last matmul, landing before W2T needs them).
Biases are zeros by problem spec; bo is applied on host if nonzero.
"""
import sys

if '/opt/trn_rl_repo' not in sys.path:
    sys.path.insert(0, '/opt/trn_rl_repo')

from contextlib import ExitStack

import numpy as np
import ml_dtypes

import concourse.bacc as bacc_mod
import concourse.mybir as mybir
import concourse.tile as tile
from concourse.bass_utils import run_bass_kernel_spmd

F32 = mybir.dt.float32
BF16 = mybir.dt.bfloat16
EXP = mybir.ActivationFunctionType.Exp
MULT = mybir.AluOpType.mult

B, S, D = 4, 2048, 1024
SQ = 1024           # queries per core
P = 128
NDC = D // P        # 8 chunks of 128 over d / d' / e / o
NKC = S // P        # 16 key chunks
NQH = SQ // 512     # 2 query column-halves (moving dim 512)
NH2 = D // 512      # 2 column-halves of a [*, D] product

LAST_RESULT = [None]
_CACHE = {}


def build_nc():
    nc = bacc_mod.Bacc("TRN2", target_bir_lowering=False, debug=False)

    xt = nc.dram_tensor("xt", [D, S], BF16, kind="ExternalInput")
    xn = nc.dram_tensor("xn", [S, D], BF16, kind="ExternalInput")
    wq = nc.dram_tensor("wq", [D, D], BF16, kind="ExternalInput")
    wk = nc.dram_tensor("wk", [D, D], BF16, kind="ExternalInput")
    wv = nc.dram_tensor("wv", [D, D], BF16, kind="ExternalInput")
    wot = nc.dram_tensor("wot", [D, D], BF16, kind="ExternalInput")
    yt = nc.dram_tensor("yt", [D, SQ], F32, kind="ExternalOutput")

    def part3(ap):  # [R, C] dram -> [128, R/128, C] (rows on partitions)
        return ap.rearrange("(o i) c -> i o c", i=P)

    with tile.TileContext(nc) as tc, ExitStack() as ctx:
        pers = ctx.enter_context(tc.tile_pool(name="pers", bufs=1))
        ones_f = pers.tile([P, P], F32)
        nc.vector.memset(ones_f[:], 1.0)
        ones_bf = pers.tile([P, P], BF16)
        nc.vector.tensor_copy(ones_bf[:], ones_f[:])
        bcast_sb = pers.tile([P, SQ], F32)
        warm_sb = pers.tile([P, P], F32)

        # 6-bank PSUM rotation shared by every accumulation chain; +2 banks
        # for the two q-halves' colsum accumulators during B1
        mps = ctx.enter_context(tc.tile_pool(name="mps", bufs=6, space="PSUM"))

        # persistents: W2T, x^T, Zq, G (left); expT, xn (right)
        w2tp = ctx.enter_context(tc.tile_pool(name="w2t", bufs=1))
        w2t_sb = w2tp.tile([P, NDC, D], BF16)      # 16 KB/part
        xtp = ctx.enter_context(tc.tile_pool(name="xtp", bufs=1))
        xt_sb = xtp.tile([P, NDC, S], BF16)        # 32 KB/part
        zqp = ctx.enter_context(tc.tile_pool(name="zqp", bufs=1))
        zq_sb = zqp.tile([P, NDC, SQ], BF16)       # 16 KB/part
        gp = ctx.enter_context(tc.tile_pool(name="gp", bufs=1))
        g_sb = gp.tile([P, NDC, SQ], BF16)         # 16 KB/part
        yp = ctx.enter_context(tc.tile_pool(name="yp", bufs=3))
        epool = ctx.enter_context(
            tc.tile_pool(name="expt", bufs=1, side="right"))
        expt_sb = epool.tile([P, NKC, SQ], BF16)   # 32 KB/part
        xnp = ctx.enter_context(
            tc.tile_pool(name="xnp", bufs=1, side="right"))
        xn_sb = xnp.tile([P, NKC, D], BF16)        # 32 KB/part

        # ~4.3us of dummy matmuls on ones_bf while the first weight chunks
        # stream in: keeps the PE busy from ~8us so the HAM throttle reaches
        # K=8/8 before the first real matmul
        ps_w = mps.tile([P, P], F32, tag="ps", name="warm")
        for i in range(40):
            nc.tensor.matmul(ps_w[:], ones_bf[:], ones_bf[:],
                             start=(i == 0), stop=(i == 39))
        nc.vector.tensor_copy(warm_sb[:], ps_w[:])

        def chain_waves(chains, lhs_of, rhs_of, out_of, nacc, tagbase):
            # waves of 6 chains; acc-step outer so 6 independent PSUM
            # accumulations ride out chunked-DMA arrival
            for w0 in range(0, len(chains), 6):
                wave = chains[w0:w0 + 6]
                ps = [mps.tile([P, 512], F32, tag="ps",
                               name=f"{tagbase}{w0}_{i}")
                      for i in range(len(wave))]
                for a in range(nacc):
                    for i, ch in enumerate(wave):
                        nc.tensor.matmul(ps[i][:], lhs_of(ch, a),
                                         rhs_of(ch, a),
                                         start=(a == 0), stop=(a == nacc - 1))
                for i, ch in enumerate(wave):
                    nc.vector.tensor_copy(out_of(ch), ps[i][:])

        chains = [(dc, h2) for dc in range(NDC) for h2 in range(NH2)]

        with tc.tile_pool(name="mp", bufs=1) as mp:
            m_sb = mp.tile([P, NDC, D], BF16)      # 16 KB/part
            # ---- A1: M = Wq^T Wk ----
            with tc.tile_pool(name="wqk", bufs=1) as wqk:
                wq_sb = wqk.tile([P, NDC, D], BF16)
                wk_sb = wqk.tile([P, NDC, D], BF16)
                for c in range(NDC):  # wq on sync, wk on scalar: chunk
                    nc.sync.dma_start(wq_sb[:, c, :],  # pairs land 2x faster
                                      wq[c * P:(c + 1) * P, :])
                    nc.scalar.dma_start(wk_sb[:, c, :],
                                        wk[c * P:(c + 1) * P, :])
                # x^T and xn stream behind the weights on both queues
                for sh in range(2):
                    nc.sync.dma_start(
                        xt_sb[:, :, sh * 1024:sh * 1024 + 512],
                        part3(xt[:, sh * 1024:sh * 1024 + 512]))
                    nc.scalar.dma_start(
                        xt_sb[:, :, sh * 1024 + 512:(sh + 1) * 1024],
                        part3(xt[:, sh * 1024 + 512:(sh + 1) * 1024]))
                nc.sync.dma_start(xn_sb[:], part3(xn))
                # M[d, d'] = sum_e1 Wq[e1, d] Wk[e1, d']
                chain_waves(
                    chains,
                    lambda ch, a: wq_sb[:, a, ch[0] * P:(ch[0] + 1) * P],
                    lambda ch, a: wk_sb[:, a, ch[1] * 512:(ch[1] + 1) * 512],
                    lambda ch: m_sb[:, ch[0], ch[1] * 512:(ch[1] + 1) * 512],
                    NDC, "mm")

            # ---- A2: Zq = M^T @ xq (xq = xt cols 0:1024) ----
            chain_waves(
                chains,
                lambda ch, a: m_sb[:, a, ch[0] * P:(ch[0] + 1) * P],
                lambda ch, a: xt_sb[:, a, ch[1] * 512:(ch[1] + 1) * 512],
                lambda ch: zq_sb[:, ch[0], ch[1] * 512:(ch[1] + 1) * 512],
                NDC, "zq")

            # ---- A3: W2T = Wv^T Wo^T (wv/wo reuse wq/wk's bytes; DMA is
            # zone-gated on M's last matmul, landing before W2T starts) ----
            with tc.tile_pool(name="wvo", bufs=1) as wvo:
                wv_sb = wvo.tile([P, NDC, D], BF16)
                wo_sb = wvo.tile([P, NDC, D], BF16)
                for c in range(NDC):
                    nc.sync.dma_start(wv_sb[:, c, :],
                                      wv[c * P:(c + 1) * P, :])
                    nc.scalar.dma_start(wo_sb[:, c, :],
                                        wot[c * P:(c + 1) * P, :])
                # W2T[d, o] = sum_e Wv[e, d] Wo[o, e]
                chain_waves(
                    chains,
                    lambda ch, a: wv_sb[:, a, ch[0] * P:(ch[0] + 1) * P],
                    lambda ch, a: wo_sb[:, a, ch[1] * 512:(ch[1] + 1) * 512],
                    lambda ch: w2t_sb[:, ch[0],
                                      ch[1] * 512:(ch[1] + 1) * 512],
                    NDC, "w2")

        # ---- B1: scoresT -> expT (+ colsums -> recip -> bcast) ----
        with tc.tile_pool(name="sump", bufs=2, space="PSUM") as sump:
            for qh in range(NQH):
                q0 = qh * 512
                ps_sum = sump.tile([P, 512], F32, tag="pssum")
                pending = None  # colsum for chunk k deferred one chain
                for kc in range(NKC):
                    ps_s = mps.tile([P, 512], F32, tag="ps",
                                    name=f"pss{qh}_{kc}")
                    for dc in range(NDC):
                        nc.tensor.matmul(
                            ps_s[:], xt_sb[:, dc, kc * P:(kc + 1) * P],
                            zq_sb[:, dc, q0:q0 + 512],
                            start=(dc == 0), stop=(dc == NDC - 1))
                    nc.scalar.activation(
                        expt_sb[:, kc, q0:q0 + 512], ps_s[:], EXP,
                        scale=1.0 / 32.0)
                    if pending is not None:
                        nc.tensor.matmul(
                            ps_sum[:], ones_bf[:],
                            expt_sb[:, pending, q0:q0 + 512],
                            start=(pending == 0), stop=False)
                    pending = kc
                nc.tensor.matmul(
                    ps_sum[:], ones_bf[:],
                    expt_sb[:, pending, q0:q0 + 512],
                    start=False, stop=True)
                # sums replicated on every partition -> full-width recip
                nc.vector.reciprocal(bcast_sb[:, q0:q0 + 512], ps_sum[:])

        # ---- B2: G = xn^T @ expT, normalized; ytT = W2T^T @ Gn ----
        for qh in range(NQH):
            q0 = qh * 512
            for dc in range(NDC):
                ps_g = mps.tile([P, 512], F32, tag="ps", name=f"pg{qh}_{dc}")
                for kc in range(NKC):
                    nc.tensor.matmul(
                        ps_g[:], xn_sb[:, kc, dc * P:(dc + 1) * P],
                        expt_sb[:, kc, q0:q0 + 512],
                        start=(kc == 0), stop=(kc == NKC - 1))
                nc.vector.tensor_tensor(
                    g_sb[:, dc, q0:q0 + 512], ps_g[:],
                    bcast_sb[:, q0:q0 + 512], MULT)
        for qh in range(NQH):
            q0 = qh * 512
            for oc in range(NDC):
                ps_y = mps.tile([P, 512], F32, tag="ps", name=f"py{qh}_{oc}")
                for dc in range(NDC):
                    nc.tensor.matmul(
                        ps_y[:], w2t_sb[:, dc, oc * P:(oc + 1) * P],
                        g_sb[:, dc, q0:q0 + 512],
                        start=(dc == 0), stop=(dc == NDC - 1))
                yst = yp.tile([P, 512], F32, tag="yst")
                nc.vector.tensor_copy(yst[:], ps_y[:])
                nc.scalar.dma_start(
                    yt[oc * P:(oc + 1) * P, q0:q0 + 512], yst[:])

    nc.compile()
    return nc


def _get_nc():
    if "nc" not in _CACHE:
        _CACHE["nc"] = build_nc()
    return _CACHE["nc"]


def kernel(x, Wq, bq, Wk, bk, Wv, bv, Wo, bo, _trace=False):
    x = np.asarray(x, dtype=np.float32)
    bf = ml_dtypes.bfloat16
    wq_b = np.ascontiguousarray(np.asarray(Wq, dtype=np.float32)).astype(bf)
    wk_b = np.ascontiguousarray(np.asarray(Wk, dtype=np.float32)).astype(bf)
    wv_b = np.ascontiguousarray(np.asarray(Wv, dtype=np.float32)).astype(bf)
    wot_b = np.ascontiguousarray(
        np.asarray(Wo, dtype=np.float32).T).astype(bf)

    in_maps = []
    for c in range(8):
        b, h = c // 2, c % 2
        xb = x[b]
        if h == 0:
            xroll = xb
        else:
            xroll = np.concatenate([xb[SQ:], xb[:SQ]], axis=0)
        xtb_ = np.ascontiguousarray(xroll.T).astype(bf)
        xnb = np.ascontiguousarray(xroll).astype(bf)
        in_maps.append({"xt": xtb_, "xn": xnb, "wq": wq_b, "wk": wk_b,
                        "wv": wv_b, "wot": wot_b})

    nc = _get_nc()
    kw = {}
    if _trace:
        kw = dict(trace=True, stitch_traces=False)
    res = run_bass_kernel_spmd(nc, in_maps, core_ids=list(range(8)), **kw)
    LAST_RESULT[0] = res

    y = np.empty((B, S, D), dtype=np.float32)
    for c in range(8):
        b, h = c // 2, c % 2
        y[b, h * SQ:(h + 1) * SQ, :] = res.results[c]["yt"].T

    bo = np.asarray(bo, dtype=np.float32)
    if bo.any():
        y = y + bo
    return y
